# revision 1
# baseline (speedup 1.0000x reference)
"""Trainium2 Bass kernel: 3x3 stride-1 pad-1 conv2d, NCHW int32 (quantized).

Contract: kernel(x, weight) takes the FULL inputs
  x      (32, 256, 56, 56) int32, values in [0, 16)
  weight (256, 256, 3, 3)  int32, values in [0, 15)
and returns the FULL (32, 256, 56, 56) int32 output of
conv2d(stride=1, padding=1), bit-exact.

Strategy
--------
Data-parallel over batch: 32 images -> 8 NeuronCores x 4 images, weights
replicated. Inputs are small non-negative ints, exactly representable in fp8
e4m3; fp8 products accumulate exactly in fp32 PSUM (max accumulator
15*14*9*256 < 2^24), so the whole computation is exact integer arithmetic.
Host converts int32 -> fp8; the conv runs as 9 shifted matmuls per output
tile with DoubleRow perf mode contracting all 256 input channels per
instruction (K = 128 partitions x 2 k-tiles; ~157 TF/s peak).

Layout: per core, x is stored padded as [c_lo=128 partitions][img][c_hi=2]
[64x57 fp8 plane]: the 56x56 image sits at rows 1..56 / cols 1..56. Row
stride 57 = 1 left-pad col + 56 pixels, so the left zero column of row r+1
doubles as the right pad of row r, making every conv tap (dy,dx) over an
8-row output block a single contiguous 456-column slice of the plane.
rhs AP [128, 2, 456] (912 fp8 moving elements <= 1024; PSUM tile 456 fp32
<= one bank; c_hi stride 3648 is 16-aligned as DoubleRow requires). Column
w=56 of each block row is padding garbage, skipped on evacuation
(PSUM -> VectorE int32 cast -> SBUF -> DMA).

Raw bacc (no TileContext) with hand-rolled semaphores: byte-precise DMA
gating lets block 0 start after only the first o-half of the weights and the
top 32 rows of image 0; throwaway matmuls on zeros warm the PE clock gate
(HAM) during the initial DMAs; the final block's evacuation is split across
engines to shorten the drain tail. Out-of-order DMA completion is handled
soundly: every DMA semaphore has at most one outstanding increment when
waited on. Host DRAM layouts keep every transfer contiguous.

Measured on trn2 (8 cores, NTFF profile): ~114 us, PE >97% of the fp8
roofline for the emitted column count, bit-exact vs the jax reference.
"""

import numpy as np
import ml_dtypes

import concourse.bacc as bacc
import concourse.mybir as mybir
from concourse import bass_utils

N_CORES = 8
NIMG = 4          # images per core
O = 256           # out channels
H = W = 56
WP = 57           # padded row stride: 1 left-pad col + 56 pixels
HP = 64           # 1 top halo + 56 rows + 1 bottom halo + margin; 64*57%16==0
PLANE = HP * WP   # 3648
RB = 8            # output rows per block
NBLK = H // RB    # 7
NCOLS = RB * WP   # 456 psum columns per block
NGRP = NIMG * 2 * NBLK  # 56 (img, o-chunk, block) groups
NPS = 8           # psum banks in rotation
NOB = 8           # output staging buffers
CUT1 = 16 * WP    # image-0 top chunk: rows <16 cover block 0 (+halo)
CUT2 = 32 * WP    # mid chunk: rows <32 cover blocks 1-2 (+halo)
F8 = ml_dtypes.float8_e4m3

_CACHED_NC = None


def _build_module():
    nc = bacc.Bacc("TRN2", target_bir_lowering=False, debug=False,
                   num_devices=N_CORES)
    # xp region stream per partition (total NIMG*2*PLANE elements):
    #   [img0 c0 rows<32 | img0 c1 rows<32 | img0 c0 rest | img0 c1 rest |
    #    img1 c0 plane | img1 c1 plane | img2 ... | img3 ...]
    xp_d = nc.dram_tensor("xp", [128, NIMG * 2 * PLANE], mybir.dt.float8e4,
                          kind="ExternalInput").ap()
    wt_d = nc.dram_tensor("wt", [2, 128, 9, 2, 128], mybir.dt.float8e4,
                          kind="ExternalInput").ap()
    # per-core output layout chosen for DMA descriptor efficiency (1792B
    # contiguous runs per partition); host reassembles to NCHW
    y_d = nc.dram_tensor("y", [NIMG, 2, NBLK, 128, RB * W], mybir.dt.int32,
                         kind="ExternalOutput").ap()

    w_sb = [nc.alloc_sbuf_tensor(f"w_sb{oc}", [128, 9, 2, 128],
                                 mybir.dt.float8e4).ap() for oc in range(2)]
    x_sb = [nc.alloc_sbuf_tensor(f"x_sb{i}", [128, 2, PLANE],
                                 mybir.dt.float8e4).ap()
            for i in range(NIMG)]
    o_sb = [nc.alloc_sbuf_tensor(f"o_sb{j}", [128, RB * W],
                                 mybir.dt.int32).ap()
            for j in range(NOB)]
    ps = [nc.alloc_psum_tensor(f"ps{j}", [128, NCOLS], mybir.dt.float32).ap()
          for j in range(NPS)]

    warm = nc.alloc_sbuf_tensor("warm", [128, 128 + NCOLS],
                                mybir.dt.float8e4).ap()
    s_zero = nc.alloc_semaphore("s_zero")
    s_w0 = nc.alloc_semaphore("s_w0")
    s_w1 = nc.alloc_semaphore("s_w1")
    s_x0a = nc.alloc_semaphore("s_x0a")
    s_x0b = nc.alloc_semaphore("s_x0b")
    s_x0c = nc.alloc_semaphore("s_x0c")
    s_x = [None] + [nc.alloc_semaphore(f"s_x{i}") for i in range(1, NIMG)]
    s_mm = nc.alloc_semaphore("s_mm")
    s_cast = nc.alloc_semaphore("s_cast")
    s_outb = [nc.alloc_semaphore(f"s_outb{j}") for j in range(NOB)]
    s_obuf = nc.alloc_semaphore("s_obuf")
    s_lc = [nc.alloc_semaphore("s_lc0"), nc.alloc_semaphore("s_lc1")]
    s_last = nc.alloc_semaphore("s_last")

    def grp(k):
        return k // (2 * NBLK), (k // NBLK) % 2, k % NBLK

    # ---- Sync engine: loads in critical-path order, then out-DMAs ---------
    nc.sync.dma_start(w_sb[0][:], wt_d[0]).then_inc(s_w0, 16)
    # image-0 row chunks, each covering both c_hi planes in one DMA
    # (DRAM contiguous, SBUF two runs)
    nc.sync.dma_start(x_sb[0][:, :, 0:CUT1],
                      xp_d[:, 0:2 * CUT1].rearrange("p (c t) -> p c t", c=2)
                      ).then_inc(s_x0a, 16)
    nc.sync.dma_start(x_sb[0][:, :, CUT1:CUT2],
                      xp_d[:, 2 * CUT1:2 * CUT2].rearrange(
                          "p (c t) -> p c t", c=2)).then_inc(s_x0b, 16)
    nc.sync.dma_start(x_sb[0][:, :, CUT2:PLANE],
                      xp_d[:, 2 * CUT2:2 * PLANE].rearrange(
                          "p (c t) -> p c t", c=2)).then_inc(s_x0c, 16)
    nc.sync.dma_start(w_sb[1][:], wt_d[1]).then_inc(s_w1, 16)
    for i in range(1, NIMG):
        nc.sync.dma_start(
            x_sb[i][:],
            xp_d[:, 2 * PLANE * i:2 * PLANE * (i + 1)].rearrange(
                "p (c t) -> p c t", c=2)).then_inc(s_x[i], 16)

    for k in range(NGRP):
        img, oc, b = grp(k)
        if k >= NOB:
            # observe slot completion; echo to DVE for o_sb reuse
            nc.sync.wait_ge(s_outb[k % NOB], 16 * (k // NOB))
            nc.sync.nop().then_inc(s_obuf, 1)
        if k == NGRP - 1:
            # final block: cast+store split in half across engines
            half = RB * W // 2
            nc.sync.wait_ge(s_lc[0], 1)
            nc.sync.dma_start(y_d[img, oc, b][:, 0:half],
                              o_sb[k % NOB][:, 0:half]).then_inc(
                s_outb[k % NOB], 16)
            nc.scalar.wait_ge(s_lc[1], 1)
            nc.scalar.dma_start(y_d[img, oc, b][:, half:],
                                o_sb[k % NOB][:, half:]).then_inc(s_last, 16)
        else:
            nc.sync.wait_ge(s_cast, k + 1)
            nc.sync.dma_start(y_d[img, oc, b], o_sb[k % NOB][:]).then_inc(
                s_outb[k % NOB], 16)
    for j in range(NOB):
        njobs = (NGRP - j + NOB - 1) // NOB
        nc.sync.wait_ge(s_outb[j], 16 * njobs)
    nc.sync.wait_ge(s_last, 16)

    # ---- GpSimd: zero the warmup operands ---------------------------------
    nc.gpsimd.memset(warm[:], 0.0).then_inc(s_zero, 1)

    # ---- Tensor engine ----------------------------------------------------
    # Warm the PE clock gate (HAM) with throwaway matmuls on zeros while the
    # input DMAs are in flight; ~9 cold matmuls cross the 3.4us activity
    # window so the real matmuls below start at 2.4 GHz. start=True each, so
    # the junk never accumulates into anything group 0 keeps.
    nc.tensor.wait_ge(s_zero, 1)
    for _ in range(10):
        nc.tensor.matmul(ps[NPS - 1][:], lhsT=warm[:, 0:128],
                         rhs=warm[:, 128:128 + NCOLS], start=True, stop=True)
    nc.tensor.wait_ge(s_w0, 16)
    nc.tensor.wait_ge(s_x0a, 16)
    for k in range(NGRP):
        img, oc, b = grp(k)
        if img == 0 and oc == 0 and b == 1:
            nc.tensor.wait_ge(s_x0b, 16)
        if img == 0 and oc == 0 and b == 3:
            nc.tensor.wait_ge(s_x0c, 16)
        if img == 0 and oc == 1 and b == 0:
            nc.tensor.wait_ge(s_w1, 16)
        if img >= 1 and oc == 0 and b == 0:
            nc.tensor.wait_ge(s_x[img], 16)
        if k >= NPS and k % 4 == 0:
            # coarse PSUM WAR: groups k..k+3 reuse banks of k-8..k-5
            nc.tensor.wait_ge(s_cast, k - 4)
        for tap in range(9):
            dy, dx = tap // 3 - 1, tap % 3 - 1
            base = (b * RB + 1 + dy) * WP + 1 + dx
            mm = nc.tensor.matmul(
                ps[k % NPS][:],
                lhsT=w_sb[oc][:, tap],
                rhs=x_sb[img][:, :, base:base + NCOLS],
                start=(tap == 0), stop=(tap == 8),
                perf_mode=mybir.MatmulPerfMode.DoubleRow)
        mm.then_inc(s_mm, 1)

    # ---- Vector engine: PSUM -> int32 SBUF --------------------------------
    for k in range(NGRP):
        nc.vector.wait_ge(s_mm, k + 1)
        if k >= NOB:
            nc.vector.wait_ge(s_obuf, k - (NOB - 1))
        src = ps[k % NPS].rearrange("p (r w) -> p r w", w=WP)
        dst = o_sb[k % NOB].rearrange("p (r w) -> p r w", w=W)
        if k == NGRP - 1:
            hr = RB // 2
            nc.vector.tensor_copy(dst[:, 0:hr], src[:, 0:hr, 0:W]).then_inc(
                s_lc[0], 1)
            nc.vector.tensor_copy(dst[:, hr:], src[:, hr:RB, 0:W]).then_inc(
                s_lc[1], 1)
        else:
            nc.vector.tensor_copy(o_sb[k % NOB][:], src[:, :, 0:W]).then_inc(
                s_cast, 1)

    nc.sync.drain()
    nc.all_engine_barrier()
    nc.compile()
    return nc


def _get_nc():
    global _CACHED_NC
    if _CACHED_NC is None:
        _CACHED_NC = _build_module()
    return _CACHED_NC


def _prep_inputs(x: np.ndarray, weight: np.ndarray):
    """Host-side conversion to the kernel's DRAM layouts (exact for the
    quantized value ranges)."""
    xr = x.astype(np.float32).astype(F8).reshape(N_CORES, NIMG, 2, 128, H, W)
    pad = np.zeros((N_CORES, 128, NIMG, 2, HP, WP), F8)
    pad[:, :, :, :, 1:H + 1, 1:W + 1] = xr.transpose(0, 3, 1, 2, 4, 5)
    pad = pad.reshape(N_CORES, 128, NIMG, 2, PLANE)
    xp_all = np.empty((N_CORES, 128, NIMG * 2 * PLANE), F8)
    # image 0: three row chunks, each [c0 | c1]
    xp_all[:, :, 0:2 * CUT1] = pad[:, :, 0, :, 0:CUT1].reshape(
        N_CORES, 128, -1)
    xp_all[:, :, 2 * CUT1:2 * CUT2] = pad[:, :, 0, :, CUT1:CUT2].reshape(
        N_CORES, 128, -1)
    xp_all[:, :, 2 * CUT2:2 * PLANE] = pad[:, :, 0, :, CUT2:PLANE].reshape(
        N_CORES, 128, -1)
    # images 1..3: [c0 plane | c1 plane]
    xp_all[:, :, 2 * PLANE:] = pad[:, :, 1:].reshape(N_CORES, 128, -1)

    wt = weight.astype(np.float32).astype(F8)
    # (O, C, 3, 3) -> [oc][c_lo][tap][c_hi][o_in_half]
    wt = wt.reshape(2, 128, 2, 128, 3, 3).transpose(0, 3, 4, 5, 2, 1)
    wt2 = np.ascontiguousarray(wt.reshape(2, 128, 9, 2, 128))
    return xp_all, wt2


def run_on_device(x: np.ndarray, weight: np.ndarray, **run_kwargs):
    """Build in_maps, run the SPMD kernel on 8 cores, return (y, results)."""
    nc = _get_nc()
    xp_all, wt2 = _prep_inputs(x, weight)
    in_maps = [{"xp": xp_all[c], "wt": wt2} for c in range(N_CORES)]
    res = bass_utils.run_bass_kernel_spmd(
        nc, in_maps, core_ids=list(range(N_CORES)), **run_kwargs)
    y = np.concatenate(
        [res.results[c]["y"].reshape(NIMG, 2, NBLK, 128, RB, W)
         .transpose(0, 1, 3, 2, 4, 5).reshape(NIMG, O, H, W)
         for c in range(N_CORES)], axis=0)
    return y, res


def kernel(x: np.ndarray, weight: np.ndarray) -> np.ndarray:
    y, _ = run_on_device(np.asarray(x), np.asarray(weight))
    return y



# revision 3
# speedup vs baseline: 1.1185x; 1.1185x over previous
"""Trainium2 Bass kernel: 3x3 stride-1 pad-1 conv2d, NCHW int32 (quantized).

Contract: kernel(x, weight) takes the FULL inputs
  x      (32, 256, 56, 56) int32, values in [0, 16)
  weight (256, 256, 3, 3)  int32, values in [0, 15)
and returns the FULL (32, 256, 56, 56) int32 output of
conv2d(stride=1, padding=1), bit-exact.

Strategy
--------
Data-parallel over batch: 32 images -> 8 NeuronCores x 4 images, weights
replicated. Inputs are small non-negative ints, exactly representable in fp8
e4m3; fp8 products accumulate exactly in fp32 PSUM (max accumulator
15*14*9*256 < 2^24), so the whole computation is exact integer arithmetic.
The conv runs as 9 shifted matmuls per output tile with DoubleRow perf mode
contracting all 256 input channels per instruction.

Layout: per core, x is stored padded as [c_lo=128 partitions][img][c_hi=2]
[64x57 fp8 plane]: the 56x56 image sits at rows 1..56 / cols 1..56. Row
stride 57 = 1 left-pad col + 56 pixels, so the left zero column of row r+1
doubles as the right pad of row r, making every conv tap (dy,dx) over an
8-row output block a single contiguous 456-column slice of the plane.

Weight-stationary schedule: the 56 (img, oc, block) output groups are
processed as 14 sweeps of (oc, block) x 4 images. Within a sweep, taps are
outer: each tap's weights are loaded once (self-loading matmul) and the
other 3 images' matmuls reuse them (InstMatmult.ldweights=False), cutting
LDWEIGHTS from 504 to 126. Each sweep accumulates 4 PSUM banks (quad
selected by sweep parity); evacuation (PSUM -> int32 cast -> SBUF staging)
overlaps the next sweep, and one merged DMA per sweep stores all 4 groups.
Input x streams in 16-row chunks spanning all 4 images on the scalar DMA
queue; outputs go on the sync queue. Throwaway matmuls on garbage warm the
PE clock gate (HAM) during the initial DMAs; the final sweep's evacuation
is split across vector/scalar engines with split stores to shorten the
drain tail.
"""

import numpy as np
import ml_dtypes

import concourse.bacc as bacc
import concourse.mybir as mybir
from concourse import bass_utils

N_CORES = 8
NIMG = 4          # images per core
O = 256           # out channels
H = W = 56
WP = 57           # padded row stride: 1 left-pad col + 56 pixels
HP = 64           # 1 top halo + 56 rows + 1 bottom halo + margin
PLANE = HP * WP   # 3648
RB = 8            # output rows per block
NBLK = H // RB    # 7
NCOLS = RB * WP   # 456 psum columns per block
NSW = 2 * NBLK    # 14 sweeps of (oc, block)
CROWS = 16        # x streamed in 16-row chunks across all images
NCH = HP // CROWS                 # 4 chunks
CH_P = CROWS * WP                 # 912 plane elems per (img,c) per chunk
CH_T = NIMG * 2 * CH_P            # 7296 dram elems per partition per chunk
SEG = RB * W                      # 448 output cols per group
F8 = ml_dtypes.float8_e4m3
DR = mybir.MatmulPerfMode.DoubleRow

_CACHED_NC = None


def _build_module():
    nc = bacc.Bacc("TRN2", target_bir_lowering=False, debug=False,
                   num_devices=N_CORES)
    # xp per partition: [chunk k][img][c_hi][rows 16k..16k+16 of plane]
    xp_d = nc.dram_tensor("xp", [128, NCH * CH_T], mybir.dt.float8e4,
                          kind="ExternalInput").ap()
    wt_d = nc.dram_tensor("wt", [2, 128, 9, 2, 128], mybir.dt.float8e4,
                          kind="ExternalInput").ap()
    # sweep-major output: one contiguous [NIMG, 128, 448] tile per sweep
    y_d = nc.dram_tensor("y", [2, NBLK, NIMG, 128, SEG], mybir.dt.int32,
                         kind="ExternalOutput").ap()

    w_sb = [nc.alloc_sbuf_tensor(f"w_sb{oc}", [128, 9, 2, 128],
                                 mybir.dt.float8e4).ap() for oc in range(2)]
    x_all = nc.alloc_sbuf_tensor("x_all", [128, NIMG, 2, PLANE],
                                 mybir.dt.float8e4).ap()
    ob = [nc.alloc_sbuf_tensor(f"ob{p}", [128, NIMG * SEG],
                               mybir.dt.int32).ap() for p in range(2)]
    ps = [nc.alloc_psum_tensor(f"ps{j}", [128, NCOLS], mybir.dt.float32).ap()
          for j in range(8)]
    # warmup operands: garbage fp8 is fine, results are discarded
    warm = nc.alloc_sbuf_tensor("warm", [128, 128 + NCOLS],
                                mybir.dt.float8e4).ap()

    s_w0 = nc.alloc_semaphore("s_w0")
    s_w1 = nc.alloc_semaphore("s_w1")
    s_xc = [nc.alloc_semaphore(f"s_xc{k}") for k in range(NCH)]
    s_mm = nc.alloc_semaphore("s_mm")      # +1 per completed group (tap 8)
    s_cast = nc.alloc_semaphore("s_cast")  # +1 per evacuated group
    s_outb = [nc.alloc_semaphore(f"s_outb{p}") for p in range(2)]
    s_lc = [nc.alloc_semaphore(f"s_lc{i}") for i in range(NIMG)]
    s_last = [nc.alloc_semaphore(f"s_last{h}") for h in range(2)]

    # ---- Scalar engine: input DMAs in need order ------------------------
    nc.scalar.dma_start(w_sb[0][:], wt_d[0]).then_inc(s_w0, 16)
    for k in range(NCH):
        if k == 1:
            nc.scalar.dma_start(w_sb[1][:], wt_d[1]).then_inc(s_w1, 16)
        nc.scalar.dma_start(
            x_all[:, :, :, CH_P * k:CH_P * (k + 1)],
            xp_d[:, CH_T * k:CH_T * (k + 1)].rearrange(
                "p (i c t) -> p i c t", i=NIMG, c=2)).then_inc(s_xc[k], 16)

    # ---- Tensor engine --------------------------------------------------
    # Warm the PE clock gate (HAM) with throwaway matmuls while the input
    # DMAs are in flight (~3.4us activity ramps the PE to 2.4 GHz).
    for _ in range(8):
        nc.tensor.matmul(ps[7][:], lhsT=warm[:, 0:128],
                         rhs=warm[:, 128:128 + NCOLS], start=True, stop=True)

    # chunk gate per block: block b reads plane rows 8b .. 8b+9
    blk_chunk = [max(0, (8 * b + 9) // CROWS) for b in range(NBLK)]
    for s in range(NSW):
        oc, b = s // NBLK, s % NBLK
        q = 4 * (s % 2)
        if s == 0:
            nc.tensor.wait_ge(s_w0, 16)
        if s == NBLK:
            nc.tensor.wait_ge(s_w1, 16)
        if oc == 0 and (b == 0 or blk_chunk[b] != blk_chunk[b - 1]):
            nc.tensor.wait_ge(s_xc[blk_chunk[b]], 16)
        if s >= 2:
            # PSUM WAR: this quad was last used by sweep s-2
            nc.tensor.wait_ge(s_cast, 4 * s - 4)
        for tap in range(9):
            dy, dx = tap // 3 - 1, tap % 3 - 1
            base = (b * RB + 1 + dy) * WP + 1 + dx
            for i in range(NIMG):
                mm = nc.tensor.matmul(
                    ps[q + i][:],
                    lhsT=w_sb[oc][:, tap],
                    rhs=x_all[:, i, :, base:base + NCOLS],
                    start=(tap == 0), stop=(tap == 8),
                    perf_mode=DR, skip_group_check=True)
                if i > 0:
                    # reuse the weights the i==0 matmul loaded
                    mm.ins.ldweights = False
                if tap == 8:
                    mm.then_inc(s_mm, 1)

    # ---- Vector engine: PSUM -> int32 SBUF staging ----------------------
    def cast_group(eng, s, i, sem, inc):
        q = 4 * (s % 2)
        src = ps[q + i].rearrange("p (r w) -> p r w", w=WP)
        dst = ob[s % 2].rearrange("p (i r w) -> p i r w", i=NIMG, w=W)
        op = getattr(eng, "tensor_copy", None) or eng.copy
        op(dst[:, i], src[:, 0:RB, 0:W]).then_inc(sem, inc)

    for s in range(NSW - 1):
        if s >= 2:
            nc.vector.wait_ge(s_outb[s % 2], 16 * (s // 2))
        for i in range(NIMG):
            nc.vector.wait_ge(s_mm, 4 * s + i + 1)
            cast_group(nc.vector, s, i, s_cast, 1)
    # final sweep: split across vector (i0, i1, i3) and scalar (i2)
    sl = NSW - 1
    nc.vector.wait_ge(s_outb[1], 16 * (sl // 2))
    for i in (0, 1):
        nc.vector.wait_ge(s_mm, 4 * sl + i + 1)
        cast_group(nc.vector, sl, i, s_lc[i], 1)
    nc.scalar.wait_ge(s_outb[1], 16 * (sl // 2))
    nc.scalar.wait_ge(s_mm, 4 * sl + 3)
    cast_group(nc.scalar, sl, 2, s_lc[2], 1)
    nc.vector.wait_ge(s_mm, 4 * sl + 4)
    cast_group(nc.vector, sl, 3, s_lc[3], 1)

    # ---- Sync engine: merged output stores ------------------------------
    for s in range(NSW - 1):
        oc, b = s // NBLK, s % NBLK
        nc.sync.wait_ge(s_cast, 4 * s + 4)
        nc.sync.dma_start(
            y_d[oc, b].rearrange("i p t -> p i t"),
            ob[s % 2].rearrange("p (i t) -> p i t", i=NIMG)).then_inc(
            s_outb[s % 2], 16)
    # final sweep: two half stores on sync and scalar
    oc, b = sl // NBLK, sl % NBLK
    dst = y_d[oc, b].rearrange("i p t -> p i t")
    src = ob[1].rearrange("p (i t) -> p i t", i=NIMG)
    nc.sync.wait_ge(s_lc[0], 1)
    nc.sync.wait_ge(s_lc[1], 1)
    nc.sync.dma_start(dst[:, 0:2], src[:, 0:2]).then_inc(s_last[0], 16)
    nc.scalar.wait_ge(s_lc[3], 1)
    nc.scalar.dma_start(dst[:, 2:4], src[:, 2:4]).then_inc(s_last[1], 16)

    nc.sync.wait_ge(s_outb[0], 16 * 7)   # sweeps 0,2,..,12
    nc.sync.wait_ge(s_outb[1], 16 * 6)   # sweeps 1,3,..,11
    nc.sync.wait_ge(s_last[0], 16)
    nc.sync.wait_ge(s_last[1], 16)

    nc.sync.drain()
    nc.all_engine_barrier()
    nc.compile()
    return nc


def _get_nc():
    global _CACHED_NC
    if _CACHED_NC is None:
        _CACHED_NC = _build_module()
    return _CACHED_NC


def _prep_inputs(x: np.ndarray, weight: np.ndarray):
    """Host-side conversion to the kernel's DRAM layouts (exact for the
    quantized value ranges)."""
    xr = x.astype(np.float32).astype(F8).reshape(N_CORES, NIMG, 2, 128, H, W)
    pad = np.zeros((N_CORES, 128, NIMG, 2, HP, WP), F8)
    pad[:, :, :, :, 1:H + 1, 1:W + 1] = xr.transpose(0, 3, 1, 2, 4, 5)
    # [core, part, img, c, 64, 57] -> chunk-major [core, part, k, img, c, 912]
    ch = pad.reshape(N_CORES, 128, NIMG, 2, NCH, CH_P).transpose(0, 1, 4, 2, 3, 5)
    xp_all = np.ascontiguousarray(ch).reshape(N_CORES, 128, NCH * CH_T)

    wt = weight.astype(np.float32).astype(F8)
    # (O, C, 3, 3) -> [oc][c_lo][tap][c_hi][o_in_half]
    wt = wt.reshape(2, 128, 2, 128, 3, 3).transpose(0, 3, 4, 5, 2, 1)
    wt2 = np.ascontiguousarray(wt.reshape(2, 128, 9, 2, 128))
    return xp_all, wt2


def run_on_device(x: np.ndarray, weight: np.ndarray, **run_kwargs):
    """Build in_maps, run the SPMD kernel on 8 cores, return (y, results)."""
    nc = _get_nc()
    xp_all, wt2 = _prep_inputs(x, weight)
    in_maps = [{"xp": xp_all[c], "wt": wt2} for c in range(N_CORES)]
    res = bass_utils.run_bass_kernel_spmd(
        nc, in_maps, core_ids=list(range(N_CORES)), **run_kwargs)
    y = np.concatenate(
        [res.results[c]["y"].reshape(2, NBLK, NIMG, 128, RB, W)
         .transpose(2, 0, 3, 1, 4, 5).reshape(NIMG, O, H, W)
         for c in range(N_CORES)], axis=0)
    return y, res


def kernel(x: np.ndarray, weight: np.ndarray) -> np.ndarray:
    y, _ = run_on_device(np.asarray(x), np.asarray(weight))
    return y


# revision 7
# speedup vs baseline: 1.1442x; 1.0230x over previous
"""Trainium2 Bass kernel: 3x3 stride-1 pad-1 conv2d, NCHW int32 (quantized).

Contract: kernel(x, weight) takes the FULL inputs
  x      (32, 256, 56, 56) int32, values in [0, 16)
  weight (256, 256, 3, 3)  int32, values in [0, 15)
and returns the FULL (32, 256, 56, 56) int32 output of
conv2d(stride=1, padding=1), bit-exact.

Strategy
--------
Data-parallel over batch: 32 images -> 8 NeuronCores x 4 images, weights
replicated. Inputs are small non-negative ints, exactly representable in fp8
e4m3; fp8 products accumulate exactly in fp32 PSUM (max accumulator
15*14*9*256 < 2^24), so the whole computation is exact integer arithmetic.
The conv runs as 9 shifted matmuls per output tile with DoubleRow perf mode
contracting all 256 input channels per instruction.

Layout: per core, x is stored padded as [c_lo=128 partitions][img][c_hi=2]
[64x57 fp8 plane]: the 56x56 image sits at rows 1..56 / cols 1..56. Row
stride 57 = 1 left-pad col + 56 pixels, so the left zero column of row r+1
doubles as the right pad of row r. Each conv tap over an 8-row output block
is a 3-free-dim moving AP [k-tile=2][row=8][col=56] (ISA sizes [56,8,2]) —
448 emitted columns per matmul with zero padding waste, and plainly
contiguous [128,448] PSUM tiles.

Weight-stationary schedule: the 56 (img, oc, block) output groups are
processed as 14 sweeps of (oc, block) x 4 images. Within a sweep, taps are
outer: each tap's weights are loaded once (self-loading matmul) and the
other 3 images' matmuls reuse them (InstMatmult.ldweights=False), cutting
LDWEIGHTS from 504 to 126 and hiding the weight-load entirely — measured
steady state is ~the 448-column stream floor. Each sweep accumulates 4 PSUM
banks (quad selected by sweep parity); evacuation (PSUM -> int32 cast ->
SBUF staging) overlaps the next sweep, and one merged DMA per sweep stores
all 4 groups. Input x streams on the scalar DMA queue in 16-row chunks (the
first chunk split per-image so sweep 0 starts as soon as image 0 lands);
outputs go on the sync queue. Throwaway matmuls on garbage warm the PE
clock gate (HAM) during the initial DMAs; the final sweep's evacuation is
split across vector/scalar engines with quarter stores on four queues to
shorten the drain tail.
"""

import numpy as np
import ml_dtypes

import concourse.bacc as bacc
import concourse.mybir as mybir
from concourse import bass_utils

N_CORES = 8
NIMG = 4          # images per core
O = 256           # out channels
H = W = 56
WP = 57           # padded row stride: 1 left-pad col + 56 pixels
HP = 64           # 1 top halo + 56 rows + 1 bottom halo + margin
PLANE = HP * WP   # 3648
RB = 8            # output rows per block
NBLK = H // RB    # 7
NSW = 2 * NBLK    # 14 sweeps of (oc, block)
CROWS = 16        # x streamed in 16-row chunks across all images
NCH = HP // CROWS                 # 4 chunks
CH_P = CROWS * WP                 # 912 plane elems per (img,c) per chunk
CH_T = NIMG * 2 * CH_P            # 7296 dram elems per partition per chunk
SEG = RB * W                      # 448 output cols per group
F8 = ml_dtypes.float8_e4m3
DR = mybir.MatmulPerfMode.DoubleRow

_CACHED_NC = None


def _build_module():
    nc = bacc.Bacc("TRN2", target_bir_lowering=False, debug=False,
                   num_devices=N_CORES)
    # xp per partition: [chunk k][img][c_hi][rows 16k..16k+16 of plane]
    xp_d = nc.dram_tensor("xp", [128, NCH * CH_T], mybir.dt.float8e4,
                          kind="ExternalInput").ap()
    wt_d = nc.dram_tensor("wt", [2, 128, 9, 2, 128], mybir.dt.float8e4,
                          kind="ExternalInput").ap()
    # sweep-major output: one contiguous [NIMG, 128, 448] tile per sweep
    y_d = nc.dram_tensor("y", [2, NBLK, NIMG, 128, SEG], mybir.dt.int32,
                         kind="ExternalOutput").ap()

    w_sb = [nc.alloc_sbuf_tensor(f"w_sb{oc}", [128, 9, 2, 128],
                                 mybir.dt.float8e4).ap() for oc in range(2)]
    x_all = nc.alloc_sbuf_tensor("x_all", [128, NIMG, 2, PLANE],
                                 mybir.dt.float8e4).ap()

    ob = [nc.alloc_sbuf_tensor(f"ob{p}", [128, NIMG * SEG],
                               mybir.dt.int32).ap() for p in range(2)]
    ps = [nc.alloc_psum_tensor(f"ps{j}", [128, SEG], mybir.dt.float32).ap()
          for j in range(8)]
    # warmup operands: garbage fp8 is fine, results are discarded
    warm = nc.alloc_sbuf_tensor("warm", [128, 128 + SEG],
                                mybir.dt.float8e4).ap()

    s_w0 = nc.alloc_semaphore("s_w0")
    s_w1 = nc.alloc_semaphore("s_w1")
    s_xa = [nc.alloc_semaphore(f"s_xa{i}") for i in range(NIMG)]
    s_xc = [None] + [nc.alloc_semaphore(f"s_xc{k}") for k in range(1, NCH)]
    s_mm = nc.alloc_semaphore("s_mm")      # +1 per completed group (tap 8)
    s_cast = nc.alloc_semaphore("s_cast")  # +1 per evacuated group
    s_outb = [nc.alloc_semaphore(f"s_outb{p}") for p in range(2)]
    s_lc = [nc.alloc_semaphore(f"s_lc{i}") for i in range(NIMG)]
    s_last = [nc.alloc_semaphore(f"s_last{i}") for i in range(NIMG)]

    # ---- Scalar engine: input DMAs in need order ------------------------
    nc.scalar.dma_start(w_sb[0][:], wt_d[0]).then_inc(s_w0, 16)
    for i in range(NIMG):
        # first 16-row chunk, one image at a time so sweep 0 starts early
        nc.scalar.dma_start(
            x_all[:, i, :, 0:CH_P],
            xp_d[:, 2 * CH_P * i:2 * CH_P * (i + 1)].rearrange(
                "p (c t) -> p c t", c=2)).then_inc(s_xa[i], 16)
    for k in range(1, NCH):
        nc.scalar.dma_start(
            x_all[:, :, :, CH_P * k:CH_P * (k + 1)],
            xp_d[:, CH_T * k:CH_T * (k + 1)].rearrange(
                "p (i c t) -> p i c t", i=NIMG, c=2)).then_inc(s_xc[k], 16)
    nc.scalar.dma_start(w_sb[1][:], wt_d[1]).then_inc(s_w1, 16)

    # ---- Tensor engine --------------------------------------------------
    # Warm the PE clock gate (HAM) with throwaway matmuls while the input
    # DMAs are in flight (~3.4us activity ramps the PE to 2.4 GHz).
    for _ in range(7):
        nc.tensor.matmul(ps[7][:], lhsT=warm[:, 0:128],
                         rhs=warm[:, 128:128 + SEG], start=True, stop=True)

    # chunk gate per block: block b reads plane rows 8b .. 8b+9
    blk_chunk = [(8 * b + 9) // CROWS for b in range(NBLK)]
    for s in range(NSW):
        oc, b = s // NBLK, s % NBLK
        q = 4 * (s % 2)
        if s == 0:
            nc.tensor.wait_ge(s_w0, 16)
        if s == NBLK:
            nc.tensor.wait_ge(s_w1, 16)
        if oc == 0 and b > 0 and blk_chunk[b] != blk_chunk[b - 1]:
            nc.tensor.wait_ge(s_xc[blk_chunk[b]], 16)
        if s >= 2:
            # PSUM WAR: this quad was last used by sweep s-2
            nc.tensor.wait_ge(s_cast, 4 * s - 4)
        for tap in range(9):
            dy, dx = tap // 3 - 1, tap % 3 - 1
            base = (b * RB + 1 + dy) * WP + 1 + dx
            for i in range(NIMG):
                if s == 0 and tap == 0:
                    nc.tensor.wait_ge(s_xa[i], 16)
                rhs = x_all[:, i, :, base:base + RB * WP].rearrange(
                    "p c (r w) -> p c r w", w=WP)[:, :, :, 0:W]
                mm = nc.tensor.matmul(
                    ps[q + i][:],
                    lhsT=w_sb[oc][:, tap],
                    rhs=rhs,
                    start=(tap == 0), stop=(tap == 8),
                    perf_mode=DR, skip_group_check=True)
                if i > 0:
                    # reuse the weights the i==0 matmul loaded
                    mm.ins.ldweights = False
                if tap == 8:
                    mm.then_inc(s_mm, 1)

    # ---- Vector engine: PSUM -> int32 SBUF staging ----------------------
    def cast_group(eng, s, i, sem, inc):
        q = 4 * (s % 2)
        op = getattr(eng, "tensor_copy", None) or eng.copy
        op(ob[s % 2][:, i * SEG:(i + 1) * SEG], ps[q + i][:]).then_inc(
            sem, inc)

    for s in range(NSW - 1):
        if s >= 2:
            nc.vector.wait_ge(s_outb[s % 2], 16 * (s // 2))
        for i in range(NIMG):
            nc.vector.wait_ge(s_mm, 4 * s + i + 1)
            cast_group(nc.vector, s, i, s_cast, 1)
    # final sweep: casts split vector (i0, i1, i3) / scalar (i2)
    sl = NSW - 1
    nc.vector.wait_ge(s_outb[1], 16 * (sl // 2))
    for i in (0, 1):
        nc.vector.wait_ge(s_mm, 4 * sl + i + 1)
        cast_group(nc.vector, sl, i, s_lc[i], 1)
    nc.scalar.wait_ge(s_outb[1], 16 * (sl // 2))
    nc.scalar.wait_ge(s_mm, 4 * sl + 3)
    cast_group(nc.scalar, sl, 2, s_lc[2], 1)
    nc.vector.wait_ge(s_mm, 4 * sl + 4)
    cast_group(nc.vector, sl, 3, s_lc[3], 1)

    # ---- Sync engine: merged output stores ------------------------------
    for s in range(NSW - 1):
        oc, b = s // NBLK, s % NBLK
        nc.sync.wait_ge(s_cast, 4 * s + 4)
        nc.sync.dma_start(
            y_d[oc, b].rearrange("i p t -> p i t"),
            ob[s % 2].rearrange("p (i t) -> p i t", i=NIMG)).then_inc(
            s_outb[s % 2], 16)
    # final sweep: four quarter stores on four queues
    oc, b = sl // NBLK, sl % NBLK
    obl = ob[1].rearrange("p (i t) -> p i t", i=NIMG)
    nc.sync.wait_ge(s_lc[0], 1)
    nc.sync.dma_start(y_d[oc, b, 0], obl[:, 0]).then_inc(s_last[0], 16)
    nc.gpsimd.wait_ge(s_lc[1], 1)
    nc.gpsimd.dma_start(y_d[oc, b, 1], obl[:, 1]).then_inc(s_last[1], 16)
    nc.scalar.dma_start(y_d[oc, b, 2], obl[:, 2]).then_inc(s_last[2], 16)
    nc.sync.wait_ge(s_lc[3], 1)
    nc.sync.dma_start(y_d[oc, b, 3], obl[:, 3]).then_inc(s_last[3], 16)

    nc.sync.wait_ge(s_outb[0], 16 * 7)   # sweeps 0,2,..,12
    nc.sync.wait_ge(s_outb[1], 16 * 6)   # sweeps 1,3,..,11
    for i in range(NIMG):
        nc.sync.wait_ge(s_last[i], 16)

    nc.sync.drain()
    nc.all_engine_barrier()
    nc.compile()
    return nc


def _get_nc():
    global _CACHED_NC
    if _CACHED_NC is None:
        _CACHED_NC = _build_module()
    return _CACHED_NC


def _prep_inputs(x: np.ndarray, weight: np.ndarray):
    """Host-side conversion to the kernel's DRAM layouts (exact for the
    quantized value ranges)."""
    xr = x.astype(np.float32).astype(F8).reshape(N_CORES, NIMG, 2, 128, H, W)
    pad = np.zeros((N_CORES, 128, NIMG, 2, HP, WP), F8)
    pad[:, :, :, :, 1:H + 1, 1:W + 1] = xr.transpose(0, 3, 1, 2, 4, 5)
    # [core, part, img, c, 64, 57] -> chunk-major [core, part, k, img, c, 912]
    ch = pad.reshape(N_CORES, 128, NIMG, 2, NCH, CH_P).transpose(0, 1, 4, 2, 3, 5)
    xp_all = np.ascontiguousarray(ch).reshape(N_CORES, 128, NCH * CH_T)

    wt = weight.astype(np.float32).astype(F8)
    # (O, C, 3, 3) -> [oc][c_lo][tap][c_hi][o_in_half]
    wt = wt.reshape(2, 128, 2, 128, 3, 3).transpose(0, 3, 4, 5, 2, 1)
    wt2 = np.ascontiguousarray(wt.reshape(2, 128, 9, 2, 128))
    return xp_all, wt2


def run_on_device(x: np.ndarray, weight: np.ndarray, **run_kwargs):
    """Build in_maps, run the SPMD kernel on 8 cores, return (y, results)."""
    nc = _get_nc()
    xp_all, wt2 = _prep_inputs(x, weight)
    in_maps = [{"xp": xp_all[c], "wt": wt2} for c in range(N_CORES)]
    res = bass_utils.run_bass_kernel_spmd(
        nc, in_maps, core_ids=list(range(N_CORES)), **run_kwargs)
    y = np.concatenate(
        [res.results[c]["y"].reshape(2, NBLK, NIMG, 128, RB, W)
         .transpose(2, 0, 3, 1, 4, 5).reshape(NIMG, O, H, W)
         for c in range(N_CORES)], axis=0)
    return y, res


def kernel(x: np.ndarray, weight: np.ndarray) -> np.ndarray:
    y, _ = run_on_device(np.asarray(x), np.asarray(weight))
    return y
